# revision 7
# baseline (speedup 1.0000x reference)
# Trainium2 Bass kernel for nn_Cobrablock (dense transformer block).
# Sharding: 8-way over (batch, seq-block): core c -> batch c//4, seq rows [512*(c%4), 512*(c%4)+512).
# All activations kept feature-major ([feat, row]) so matmuls chain without transposes.
# Matmuls in bf16 (fp32 accumulate in PSUM); LN stats / softmax denominators / residuals in fp32.
# One AllGather per 4-core batch group exchanges local keyh^T / valh blocks.
import math

import numpy as np
import ml_dtypes

BF16 = ml_dtypes.bfloat16

B, S, D = 2, 2048, 1024
H, K = 8, 32
HK = H * K  # 256
N_CORES = 8
ROWS = 512          # rows (queries) per core
NT = D // 128       # 8 feature tiles
EPS = 1e-5

_CACHE = {}


def _build_program():
    import concourse.bass as bass
    import concourse.mybir as mybir
    import concourse.tile as tile
    from concourse import bacc

    f32 = mybir.dt.float32
    bf16 = mybir.dt.bfloat16
    AF = mybir.ActivationFunctionType
    OP = mybir.AluOpType

    nc = bacc.Bacc("TRN2", target_bir_lowering=False, debug=False, num_devices=N_CORES)

    xT = nc.dram_tensor("xT", [D, ROWS], f32, kind="ExternalInput")
    wbig = {}
    for nm in ("w1", "w2", "w3", "wg", "wu", "wd"):
        wbig[nm] = nc.dram_tensor(nm, [D, D], bf16, kind="ExternalInput")
    wq = nc.dram_tensor("wq", [D, HK], bf16, kind="ExternalInput")
    wk = nc.dram_tensor("wk", [D, HK], bf16, kind="ExternalInput")
    wv = nc.dram_tensor("wv", [D, HK], bf16, kind="ExternalInput")
    wo = nc.dram_tensor("wo", [HK, D], bf16, kind="ExternalInput")
    bcols = nc.dram_tensor("bcols", [10, D], f32, kind="ExternalInput")
    cosT = nc.dram_tensor("cosT", [D // 2, ROWS], bf16, kind="ExternalInput")
    sinT = nc.dram_tensor("sinT", [D // 2, ROWS], bf16, kind="ExternalInput")
    outT = nc.dram_tensor("outT", [D, ROWS], f32, kind="ExternalOutput")

    with tile.TileContext(nc) as tc:
        from contextlib import ExitStack

        ctx = ExitStack()
        const = ctx.enter_context(tc.tile_pool(name="const", bufs=1))
        wts = ctx.enter_context(tc.tile_pool(name="wts", bufs=3))
        wsm = ctx.enter_context(tc.tile_pool(name="wsm", bufs=1))
        tmp = ctx.enter_context(tc.tile_pool(name="tmp", bufs=4))
        rows = ctx.enter_context(tc.tile_pool(name="rows", bufs=5))
        rbp = ctx.enter_context(tc.tile_pool(name="rbp", bufs=1))
        epool = ctx.enter_context(tc.tile_pool(name="epool", bufs=5))
        acts = ctx.enter_context(tc.tile_pool(name="acts", bufs=4))
        dram = ctx.enter_context(tc.tile_pool(name="dram", bufs=1, space="DRAM"))

        # ---- constants / inputs resident in SBUF ----
        bias_sb = const.tile([128, 10, 8], f32)
        nc.sync.dma_start(bias_sb[:], bcols.rearrange("r (o p) -> p r o", p=128))
        bhc = const.tile([128, 6], f32)  # head-bias cols: q:0,1 k:2,3 v:4,5
        nc.sync.dma_start(bhc[:], bcols[9:10, 0:768].rearrange("r (t o p) -> p (r t o)", p=128, t=3, o=2))
        bvb = const.tile([128, HK], f32)  # v-head bias broadcast over partitions
        nc.sync.dma_start(bvb[:], bcols[9:10, 512:768].to_broadcast([128, HK]))
        cos_sb = const.tile([128, 4, ROWS], bf16)
        nc.sync.dma_start(cos_sb[:], cosT.rearrange("(o p) f -> p o f", p=128))
        sin_sb = const.tile([128, 4, ROWS], bf16)
        nc.sync.dma_start(sin_sb[:], sinT.rearrange("(o p) f -> p o f", p=128))
        ones_bf = const.tile([128, 1], bf16)
        nc.vector.memset(ones_bf[:], 1.0)
        eps_sb = const.tile([1, 1], f32)
        nc.vector.memset(eps_sb[:], EPS)
        xT_sb = const.tile([128, NT, ROWS], f32)
        nc.sync.dma_start(xT_sb[:], xT.rearrange("(o p) f -> p o f", p=128))

        def col(r, kt):
            return bias_sb[:, r, kt:kt + 1]

        # weight tiles
        w_sb = {}
        for nm in ("w2", "w3", "w1"):
            w_sb[nm] = wts.tile([128, NT, D], bf16, tag="Wbig", name=f"W_{nm}")
            nc.sync.dma_start(w_sb[nm][:], wbig[nm].rearrange("(o p) c -> p o c", p=128))
        wq_sb = wsm.tile([128, NT, HK], bf16)
        nc.sync.dma_start(wq_sb[:], wq.rearrange("(o p) c -> p o c", p=128))
        wk_sb = wsm.tile([128, NT, HK], bf16)
        nc.sync.dma_start(wk_sb[:], wk.rearrange("(o p) c -> p o c", p=128))
        wv_sb = wsm.tile([128, NT, HK], bf16)
        nc.sync.dma_start(wv_sb[:], wv.rearrange("(o p) c -> p o c", p=128))
        wo_sb = wsm.tile([128, 2, D], bf16)
        nc.sync.dma_start(wo_sb[:], wo.rearrange("(o p) c -> p o c", p=128))

        h_sb = acts.tile([128, NT, ROWS], bf16, tag="act8", name="h_sb")
        lnscr = dram.tile([2, ROWS], f32)
        ln1b = const.tile([128, 2, ROWS], f32)

        # ================= LN1 =================
        with tc.tile_pool(name="ps_st", bufs=2, space="PSUM") as ps_st:
            sum_ps = ps_st.tile([1, ROWS], f32, tag="st", name="sum_ps")
            sumsq_ps = ps_st.tile([1, ROWS], f32, tag="st", name="sumsq_ps")
            for kt in range(NT):
                xbf = tmp.tile([128, ROWS], bf16, tag="t2", name="xbf")
                nc.vector.tensor_copy(out=xbf[:], in_=xT_sb[:, kt])
                sq = tmp.tile([128, ROWS], bf16, tag="t2", name="sq")
                nc.vector.tensor_tensor(sq[:], xbf[:], xbf[:], OP.mult)
                nc.tensor.matmul(sum_ps[:], ones_bf[:], xbf[:], start=(kt == 0), stop=(kt == NT - 1))
                nc.tensor.matmul(sumsq_ps[:], ones_bf[:], sq[:], start=(kt == 0), stop=(kt == NT - 1))
            mean = rows.tile([1, ROWS], f32, tag="row", name="mean")
            nc.vector.tensor_scalar_mul(mean[:], sum_ps[:], 1.0 / D)
            ex2 = rows.tile([1, ROWS], f32, tag="row", name="ex2")
            nc.vector.tensor_scalar_mul(ex2[:], sumsq_ps[:], 1.0 / D)
            msq = rows.tile([1, ROWS], f32, tag="row", name="msq")
            nc.vector.tensor_tensor(msq[:], mean[:], mean[:], OP.mult)
            var = rows.tile([1, ROWS], f32, tag="row", name="var")
            nc.vector.tensor_tensor(var[:], ex2[:], msq[:], OP.subtract)
            std = rows.tile([1, ROWS], f32, tag="row", name="std")
            nc.scalar.activation(std[:], var[:], AF.Sqrt, bias=eps_sb[:])
            rstd = rows.tile([1, ROWS], f32, tag="row", name="rstd")
            nc.vector.reciprocal(rstd[:], std[:])
            mr = rows.tile([1, ROWS], f32, tag="row", name="mr")
            nc.vector.tensor_tensor(mr[:], mean[:], rstd[:], OP.mult)
            nc.sync.dma_start(lnscr[0:1, :], rstd[:])
            nc.sync.dma_start(lnscr[1:2, :], mr[:])
        nc.sync.dma_start(ln1b[:], lnscr[None].to_broadcast([128, 2, ROWS]))
        for kt in range(NT):
            t = tmp.tile([128, ROWS], f32, tag="t4", name="lnt")
            nc.vector.tensor_tensor(t[:], xT_sb[:, kt], ln1b[:, 0], OP.mult)
            nc.vector.tensor_tensor(h_sb[:, kt], t[:], ln1b[:, 1], OP.subtract)

        # ============ QKV projections + RoPE + head projections ============
        krot = acts.tile([128, NT, ROWS], bf16, tag="act8", name="krot")
        vbf = acts.tile([128, NT, ROWS], bf16, tag="act8", name="vbf")
        qrot = acts.tile([128, NT, ROWS], bf16, tag="act8", name="qrot")
        keyT_loc = const.tile([128, 2, ROWS], bf16)
        val_loc = const.tile([128, 4, HK], bf16)
        qhT = const.tile([128, 2, ROWS], bf16)
        in_cc = dram.tile([2 * HK * ROWS], bf16)     # keyh^T block then valh block
        out_cc = dram.tile([8 * HK * ROWS], bf16)

        def big_mm(psum_t, wname, mt, rhs_sb):
            for kt in range(NT):
                nc.tensor.matmul(psum_t[:], w_sb[wname][:, kt, mt * 128:(mt + 1) * 128],
                                 rhs_sb[:, kt], start=(kt == 0), stop=(kt == NT - 1))

        def rope(psl, bias_row, out_sb):
            # psl: list of 8 psum tiles ([128,ROWS] fp32); out_sb [128,NT,ROWS] bf16
            for pt in range(4):
                c = cos_sb[:, pt]
                s = sin_sb[:, pt]
                b1 = col(bias_row, pt)
                b2 = col(bias_row, pt + 4)
                t1 = tmp.tile([128, ROWS], bf16, tag="t2", name="r1")
                nc.vector.scalar_tensor_tensor(t1[:], psl[pt][:], b1, c, OP.add, OP.mult)
                t2 = tmp.tile([128, ROWS], bf16, tag="t2", name="r2")
                nc.vector.scalar_tensor_tensor(t2[:], psl[pt + 4][:], b2, s, OP.add, OP.mult)
                nc.vector.tensor_tensor(out_sb[:, pt], t1[:], t2[:], OP.subtract)
                t3 = tmp.tile([128, ROWS], bf16, tag="t2", name="r3")
                nc.vector.scalar_tensor_tensor(t3[:], psl[pt][:], b1, s, OP.add, OP.mult)
                t4 = tmp.tile([128, ROWS], bf16, tag="t2", name="r4")
                nc.vector.scalar_tensor_tensor(t4[:], psl[pt + 4][:], b2, c, OP.add, OP.mult)
                nc.vector.tensor_tensor(out_sb[:, pt + 4], t3[:], t4[:], OP.add)

        with tc.tile_pool(name="ps_mm", bufs=8, space="PSUM") as ps_mm:
            # k = rope(h @ w2 + b2)
            kps = []
            for mt in range(NT):
                p = ps_mm.tile([128, ROWS], f32, tag="mm", name=f"kps{mt}")
                big_mm(p, "w2", mt, h_sb)
                kps.append(p)
            rope(kps, 1, krot)
            # v = h @ w3 + b3
            for mt in range(NT):
                p = ps_mm.tile([128, ROWS], f32, tag="mm", name=f"vps{mt}")
                big_mm(p, "w3", mt, h_sb)
                nc.vector.tensor_scalar_add(vbf[:, mt], p[:], col(2, mt))
            # keyh^T = (v @ k_W + bk)^T   [256, 512]
            for mt2 in range(2):
                p = ps_mm.tile([128, ROWS], f32, tag="mm", name=f"keyps{mt2}")
                for kt in range(NT):
                    nc.tensor.matmul(p[:], wk_sb[:, kt, mt2 * 128:(mt2 + 1) * 128], vbf[:, kt],
                                     start=(kt == 0), stop=(kt == NT - 1))
                nc.vector.tensor_scalar_add(keyT_loc[:, mt2], p[:], bhc[:, 2 + mt2:3 + mt2])
            # valh = (k_rot @ v_W + bv)  natural [512, 256]
            for st in range(4):
                p = ps_mm.tile([128, HK], f32, tag="mm", name=f"valps{st}")
                for kt in range(NT):
                    nc.tensor.matmul(p[:], krot[:, kt, st * 128:(st + 1) * 128], wv_sb[:, kt],
                                     start=(kt == 0), stop=(kt == NT - 1))
                nc.vector.tensor_tensor(val_loc[:, st], p[:], bvb[:], OP.add)
            # ship local kv to DRAM and gather
            nc.sync.dma_start(in_cc[0:HK * ROWS].rearrange("(o p f) -> p o f", p=128, f=ROWS), keyT_loc[:])
            nc.sync.dma_start(in_cc[HK * ROWS:].rearrange("(st p c) -> p st c", p=128, c=HK), val_loc[:])
            nc.gpsimd.collective_compute(
                "AllGather", mybir.AluOpType.bypass,
                replica_groups=[[0, 1, 2, 3], [4, 5, 6, 7]],
                ins=[in_cc[:].opt()], outs=[out_cc[:].opt()],
            )
            # q = rope(h @ w1 + b1); qh^T = (q @ q_W + bq)^T
            qps = []
            for mt in range(NT):
                p = ps_mm.tile([128, ROWS], f32, tag="mm", name=f"qps{mt}")
                big_mm(p, "w1", mt, h_sb)
                qps.append(p)
            rope(qps, 0, qrot)
            for mt2 in range(2):
                p = ps_mm.tile([128, ROWS], f32, tag="mm", name=f"qhps{mt2}")
                for kt in range(NT):
                    nc.tensor.matmul(p[:], wq_sb[:, kt, mt2 * 128:(mt2 + 1) * 128], qrot[:, kt],
                                     start=(kt == 0), stop=(kt == NT - 1))
                nc.vector.tensor_scalar_add(qhT[:, mt2], p[:], bhc[:, mt2:mt2 + 1])

        # gathered kv into SBUF
        keyT_all = const.tile([128, 8, ROWS], bf16)   # [:, 2*cb+G, s_local]
        val_all = const.tile([128, 16, HK], bf16)     # [:, 4*cb+lt, c]
        BLK = 2 * HK * ROWS
        for cb in range(4):
            nc.sync.dma_start(keyT_all[:, 2 * cb:2 * cb + 2],
                              out_cc[cb * BLK:cb * BLK + HK * ROWS].rearrange("(o p f) -> p o f", p=128, f=ROWS))
            nc.sync.dma_start(val_all[:, 4 * cb:4 * cb + 4],
                              out_cc[cb * BLK + HK * ROWS:(cb + 1) * BLK].rearrange("(st p c) -> p st c", p=128, c=HK))

        # ================= attention =================
        aoT = const.tile([128, 2, ROWS], bf16)
        rscr = dram.tile([8, ROWS], f32)
        with tc.tile_pool(name="ps_sc", bufs=6, space="PSUM") as ps_sc, \
             tc.tile_pool(name="ps_ao", bufs=1, space="PSUM") as ps_ao, \
             tc.tile_pool(name="ps_dn", bufs=1, space="PSUM") as ps_dn:
            for G in range(2):
                ao_ps = ps_ao.tile([128, ROWS], f32, tag="ao", name=f"ao{G}")
                dn_ps = ps_dn.tile([128, ROWS], f32, tag="dn", name=f"dn{G}")
                for stg in range(16):
                    cb, lt = stg // 4, stg % 4
                    scl = []
                    for hh in range(4):
                        sc = ps_sc.tile([128, ROWS], f32, tag="sc", name=f"sc{G}_{stg}_{hh}")
                        nc.tensor.matmul(sc[:],
                                         keyT_all[32 * hh:32 * hh + 32, 2 * cb + G, lt * 128:(lt + 1) * 128],
                                         qhT[32 * hh:32 * hh + 32, G],
                                         start=True, stop=True, tile_position=(32 * hh, 0))
                        scl.append(sc)
                    for hh in range(4):
                        E = epool.tile([128, ROWS], bf16, tag="E", name=f"E{G}_{stg}_{hh}")
                        nc.scalar.activation(E[:], scl[hh][:], AF.Exp)
                        nc.tensor.matmul(ao_ps[32 * hh:32 * hh + 32, :],
                                         val_all[:, 4 * cb + lt, 128 * G + 32 * hh:128 * G + 32 * hh + 32],
                                         E[:], start=(stg == 0), stop=(stg == 15), tile_position=(0, 32 * hh))
                        nc.tensor.matmul(dn_ps[32 * hh:32 * hh + 1, :], ones_bf[:], E[:],
                                         start=(stg == 0), stop=(stg == 15), tile_position=(0, 32 * hh))
                for hh in range(4):
                    dnr = rows.tile([1, ROWS], f32, tag="row", name=f"dnr{G}_{hh}")
                    nc.vector.reciprocal(dnr[:], dn_ps[32 * hh:32 * hh + 1, :])
                    nc.sync.dma_start(rscr[4 * G + hh:4 * G + hh + 1, :], dnr[:])
                rb = rbp.tile([128, 4, ROWS], f32, tag="rb", name=f"rb{G}")
                nc.sync.dma_start(rb[:], rscr[4 * G:4 * G + 4, :][None].to_broadcast([128, 4, ROWS]))
                for hh in range(4):
                    nc.vector.tensor_tensor(aoT[32 * hh:32 * hh + 32, G],
                                            ao_ps[32 * hh:32 * hh + 32, :],
                                            rb[32 * hh:32 * hh + 32, hh], OP.mult)

        # ============ o_proj + residual + SwiGLU + LN2 + out ============
        x1 = acts.tile([128, NT, ROWS], bf16, tag="act8", name="x1")
        sg = acts.tile([128, NT, ROWS], bf16, tag="act8", name="sg")
        mm_sb = acts.tile([128, NT, ROWS], bf16, tag="act8", name="mm_sb")
        f_sb = const.tile([128, NT, ROWS], f32)
        for nm in ("wg", "wu", "wd"):
            w_sb[nm] = wts.tile([128, NT, D], bf16, tag="Wbig", name=f"W_{nm}")
            nc.sync.dma_start(w_sb[nm][:], wbig[nm].rearrange("(o p) c -> p o c", p=128))

        with tc.tile_pool(name="ps_mm2", bufs=8, space="PSUM") as ps2:
            for mt in range(NT):
                p = ps2.tile([128, ROWS], f32, tag="mm2", name=f"ops{mt}")
                for G in range(2):
                    nc.tensor.matmul(p[:], wo_sb[:, G, mt * 128:(mt + 1) * 128], aoT[:, G],
                                     start=(G == 0), stop=(G == 1))
                nc.vector.scalar_tensor_tensor(x1[:, mt], p[:], col(3, mt), xT_sb[:, mt], OP.add, OP.add)
            for mt in range(NT):
                p = ps2.tile([128, ROWS], f32, tag="mm2", name=f"gps{mt}")
                big_mm(p, "wg", mt, x1)
                nc.scalar.activation(sg[:, mt], p[:], AF.Silu, bias=col(4, mt))
            for mt in range(NT):
                p = ps2.tile([128, ROWS], f32, tag="mm2", name=f"ups{mt}")
                big_mm(p, "wu", mt, x1)
                nc.vector.scalar_tensor_tensor(mm_sb[:, mt], p[:], col(5, mt), sg[:, mt], OP.add, OP.mult)

        lnscr2 = dram.tile([2, ROWS], f32)
        ln2b = const.tile([128, 2, ROWS], f32)
        with tc.tile_pool(name="ps_mm3", bufs=6, space="PSUM") as ps3, \
             tc.tile_pool(name="ps_st2", bufs=2, space="PSUM") as ps_st2:
            sum2 = ps_st2.tile([1, ROWS], f32, tag="st2", name="sum2")
            sumsq2 = ps_st2.tile([1, ROWS], f32, tag="st2", name="sumsq2")
            for mt in range(NT):
                p = ps3.tile([128, ROWS], f32, tag="mm3", name=f"dps{mt}")
                big_mm(p, "wd", mt, mm_sb)
                nc.vector.tensor_scalar_add(f_sb[:, mt], p[:], col(6, mt))
                fbf = tmp.tile([128, ROWS], bf16, tag="t2", name="fbf")
                nc.vector.tensor_copy(out=fbf[:], in_=f_sb[:, mt])
                sqf = tmp.tile([128, ROWS], bf16, tag="t2", name="sqf")
                nc.vector.tensor_tensor(sqf[:], fbf[:], fbf[:], OP.mult)
                nc.tensor.matmul(sum2[:], ones_bf[:], fbf[:], start=(mt == 0), stop=(mt == NT - 1))
                nc.tensor.matmul(sumsq2[:], ones_bf[:], sqf[:], start=(mt == 0), stop=(mt == NT - 1))
            mean2 = rows.tile([1, ROWS], f32, tag="row", name="mean2")
            nc.vector.tensor_scalar_mul(mean2[:], sum2[:], 1.0 / D)
            ex22 = rows.tile([1, ROWS], f32, tag="row", name="ex22")
            nc.vector.tensor_scalar_mul(ex22[:], sumsq2[:], 1.0 / D)
            msq2 = rows.tile([1, ROWS], f32, tag="row", name="msq2")
            nc.vector.tensor_tensor(msq2[:], mean2[:], mean2[:], OP.mult)
            var2 = rows.tile([1, ROWS], f32, tag="row", name="var2")
            nc.vector.tensor_tensor(var2[:], ex22[:], msq2[:], OP.subtract)
            std2 = rows.tile([1, ROWS], f32, tag="row", name="std2")
            nc.scalar.activation(std2[:], var2[:], AF.Sqrt, bias=eps_sb[:])
            rstd2 = rows.tile([1, ROWS], f32, tag="row", name="rstd2")
            nc.vector.reciprocal(rstd2[:], std2[:])
            mr2 = rows.tile([1, ROWS], f32, tag="row", name="mr2")
            nc.vector.tensor_tensor(mr2[:], mean2[:], rstd2[:], OP.mult)
            nc.sync.dma_start(lnscr2[0:1, :], rstd2[:])
            nc.sync.dma_start(lnscr2[1:2, :], mr2[:])
        nc.sync.dma_start(ln2b[:], lnscr2[None].to_broadcast([128, 2, ROWS]))
        for mt in range(NT):
            t1 = tmp.tile([128, ROWS], f32, tag="t4", name="o1")
            nc.vector.tensor_tensor(t1[:], f_sb[:, mt], ln2b[:, 0], OP.mult)
            t2 = tmp.tile([128, ROWS], f32, tag="t4", name="o2")
            nc.vector.tensor_tensor(t2[:], t1[:], ln2b[:, 1], OP.subtract)
            t3 = tmp.tile([128, ROWS], f32, tag="t4", name="o3")
            nc.vector.tensor_scalar(t3[:], t2[:], col(7, mt), col(8, mt), OP.mult, OP.add)
            t4 = tmp.tile([128, ROWS], f32, tag="t4", name="o4")
            nc.vector.tensor_tensor(t4[:], t3[:], f_sb[:, mt], OP.add)
            nc.sync.dma_start(outT[mt * 128:(mt + 1) * 128, :], t4[:])
        ctx.close()

    nc.compile()
    return nc


def _prep_inputs(inputs):
    x = np.asarray(inputs["x"], np.float32)
    g1 = np.asarray(inputs["ln1_g"], np.float32)
    b1 = np.asarray(inputs["ln1_b"], np.float32)
    sc = 1.0 / math.sqrt(K)

    def fold(Wn, bn):
        W = np.asarray(inputs[Wn], np.float32)
        b = np.asarray(inputs[bn], np.float32)
        return (g1[:, None] * W).astype(BF16), (b + b1 @ W).astype(np.float32)

    w1, bw1 = fold("w1_W", "w1_b")
    w2, bw2 = fold("w2_W", "w2_b")
    w3, bw3 = fold("w3_W", "w3_b")
    wqv = (np.asarray(inputs["q_W"], np.float32).reshape(D, HK) * sc).astype(BF16)
    bq = (np.asarray(inputs["q_b"], np.float32).reshape(HK) * sc).astype(np.float32)
    wkv = np.asarray(inputs["k_W"], np.float32).reshape(D, HK).astype(BF16)
    bk = np.asarray(inputs["k_b"], np.float32).reshape(HK)
    wvv = np.asarray(inputs["v_W"], np.float32).reshape(D, HK).astype(BF16)
    bv = np.asarray(inputs["v_b"], np.float32).reshape(HK)
    wov = np.asarray(inputs["o_W"], np.float32).reshape(HK, D).astype(BF16)

    bcols = np.zeros((10, D), np.float32)
    bcols[0] = bw1
    bcols[1] = bw2
    bcols[2] = bw3
    bcols[3] = np.asarray(inputs["o_b"], np.float32)
    bcols[4] = np.asarray(inputs["gate_b"], np.float32)
    bcols[5] = np.asarray(inputs["up_b"], np.float32)
    bcols[6] = np.asarray(inputs["down_b"], np.float32)
    bcols[7] = np.asarray(inputs["ln2_g"], np.float32)
    bcols[8] = np.asarray(inputs["ln2_b"], np.float32)
    bcols[9, 0:HK] = bq
    bcols[9, HK:2 * HK] = bk
    bcols[9, 2 * HK:3 * HK] = bv

    wgv = np.asarray(inputs["gate_W"], np.float32).astype(BF16)
    wuv = np.asarray(inputs["up_W"], np.float32).astype(BF16)
    wdv = np.asarray(inputs["down_W"], np.float32).astype(BF16)

    pos = np.arange(S, dtype=np.float32)
    freq = np.power(10000.0, -np.arange(D // 2, dtype=np.float32) / (D // 2))
    ang = pos[:, None] * freq[None, :]  # [S, 512]
    cosA = np.cos(ang)
    sinA = np.sin(ang)

    in_maps = []
    for c in range(N_CORES):
        b = c // 4
        j = c % 4
        sl = slice(ROWS * j, ROWS * (j + 1))
        m = {
            "xT": np.ascontiguousarray(x[b, sl, :].T),
            "w1": w1, "w2": w2, "w3": w3,
            "wg": wgv, "wu": wuv, "wd": wdv,
            "wq": wqv, "wk": wkv, "wv": wvv, "wo": wov,
            "bcols": bcols,
            "cosT": np.ascontiguousarray(cosA[sl, :].T).astype(BF16),
            "sinT": np.ascontiguousarray(sinA[sl, :].T).astype(BF16),
        }
        in_maps.append(m)
    return in_maps


def kernel(**inputs):
    from concourse.bass_utils import run_bass_kernel_spmd

    if "nc" not in _CACHE:
        _CACHE["nc"] = _build_program()
    nc = _CACHE["nc"]
    in_maps = _prep_inputs(inputs)
    res = run_bass_kernel_spmd(nc, in_maps, list(range(N_CORES)))
    out = np.empty((B, S, D), np.float32)
    for c in range(N_CORES):
        b = c // 4
        j = c % 4
        out[b, ROWS * j:ROWS * (j + 1), :] = res.results[c]["outT"].T
    return out


# revision 13
# speedup vs baseline: 15504.8608x; 15504.8608x over previous
# Trainium2 Bass kernel for nn_Cobrablock (dense transformer block).
# Sharding: 8-way over (batch, seq-block): core c -> batch c//4, seq rows [512*(c%4), 512*(c%4)+512).
# All activations kept feature-major ([feat, row]) so matmuls chain without transposes.
# Matmuls in bf16 (fp32 accumulate in PSUM); LN stats / softmax denominators / residuals in fp32.
# One AllGather per 4-core batch group exchanges local keyh^T / valh blocks.
import math

import numpy as np
import ml_dtypes

BF16 = ml_dtypes.bfloat16

B, S, D = 2, 2048, 1024
H, K = 8, 32
HK = H * K  # 256
N_CORES = 8
ROWS = 512          # rows (queries) per core
NT = D // 128       # 8 feature tiles
EPS = 1e-5

_CACHE = {}


def _build_program(trace_sim=False):
    import concourse.bass as bass
    import concourse.mybir as mybir
    import concourse.tile as tile
    from concourse import bacc

    f32 = mybir.dt.float32
    bf16 = mybir.dt.bfloat16
    AF = mybir.ActivationFunctionType
    OP = mybir.AluOpType

    nc = bacc.Bacc("TRN2", target_bir_lowering=False, debug=False, num_devices=N_CORES)

    xT = nc.dram_tensor("xT", [D, ROWS], f32, kind="ExternalInput")
    wbig = {}
    for nm in ("w1", "w2", "w3", "wg", "wu", "wd"):
        wbig[nm] = nc.dram_tensor(nm, [D, D], bf16, kind="ExternalInput")
    wq = nc.dram_tensor("wq", [D, HK], bf16, kind="ExternalInput")
    wk = nc.dram_tensor("wk", [D, HK], bf16, kind="ExternalInput")
    wv = nc.dram_tensor("wv", [D, HK], bf16, kind="ExternalInput")
    wo = nc.dram_tensor("wo", [HK, D], bf16, kind="ExternalInput")
    bcols = nc.dram_tensor("bcols", [10, D], f32, kind="ExternalInput")
    cosT = nc.dram_tensor("cosT", [D // 2, ROWS], bf16, kind="ExternalInput")
    sinT = nc.dram_tensor("sinT", [D // 2, ROWS], bf16, kind="ExternalInput")
    outT = nc.dram_tensor("outT", [D, ROWS], f32, kind="ExternalOutput")

    with tile.TileContext(nc, trace_sim=trace_sim) as tc:
        from contextlib import ExitStack

        ctx = ExitStack()
        const = ctx.enter_context(tc.tile_pool(name="const", bufs=1))
        wts = ctx.enter_context(tc.tile_pool(name="wts", bufs=3))
        wsm = ctx.enter_context(tc.tile_pool(name="wsm", bufs=1))
        tmp = ctx.enter_context(tc.tile_pool(name="tmp", bufs=4))
        rows = ctx.enter_context(tc.tile_pool(name="rows", bufs=5))
        epool = ctx.enter_context(tc.tile_pool(name="epool", bufs=5))
        acts = ctx.enter_context(tc.tile_pool(name="acts", bufs=4))
        dram = ctx.enter_context(tc.tile_pool(name="dram", bufs=1, space="DRAM"))

        # ---- constants / inputs resident in SBUF ----
        bias_sb = const.tile([128, 10, 8], f32)
        nc.sync.dma_start(bias_sb[:], bcols.rearrange("r (o p) -> p r o", p=128))
        bhc = const.tile([128, 6], f32)  # head-bias cols: q:0,1 k:2,3 v:4,5
        nc.sync.dma_start(bhc[:], bcols[9:10, 0:768].rearrange("r (t o p) -> p (r t o)", p=128, t=3, o=2))
        bvb = const.tile([128, HK], f32)  # v-head bias broadcast over partitions
        nc.sync.dma_start(bvb[:], bcols[9:10, 512:768].to_broadcast([128, HK]))
        cos_sb = const.tile([128, 4, ROWS], bf16)
        nc.sync.dma_start(cos_sb[:], cosT.rearrange("(o p) f -> p o f", p=128))
        sin_sb = const.tile([128, 4, ROWS], bf16)
        nc.sync.dma_start(sin_sb[:], sinT.rearrange("(o p) f -> p o f", p=128))
        ones_bf = const.tile([128, 1], bf16)
        nc.vector.memset(ones_bf[:], 1.0)
        eps_sb = const.tile([1, 1], f32)
        nc.vector.memset(eps_sb[:], EPS)
        ones128 = const.tile([1, 128], f32)
        nc.vector.memset(ones128[:], 1.0)
        invd = const.tile([128, 1], bf16)
        nc.vector.memset(invd[:], 1.0 / D)
        xT_sb = const.tile([128, NT, ROWS], f32)
        for xc in range(4):
            nc.sync.dma_start(xT_sb[:, 2 * xc:2 * xc + 2],
                              xT[256 * xc:256 * (xc + 1), :].rearrange("(o p) f -> p o f", p=128))

        def col(r, kt):
            return bias_sb[:, r, kt:kt + 1]

        # weight tiles
        w_sb = {}
        for nm in ("w2", "w3", "w1"):
            w_sb[nm] = wts.tile([128, NT, D], bf16, tag="Wbig", name=f"W_{nm}")
            nc.sync.dma_start(w_sb[nm][:], wbig[nm].rearrange("(o p) c -> p o c", p=128))
        wq_sb = wsm.tile([128, NT, HK], bf16)
        nc.sync.dma_start(wq_sb[:], wq.rearrange("(o p) c -> p o c", p=128))
        wk_sb = wsm.tile([128, NT, HK], bf16)
        nc.sync.dma_start(wk_sb[:], wk.rearrange("(o p) c -> p o c", p=128))
        wv_sb = wsm.tile([128, NT, HK], bf16)
        nc.sync.dma_start(wv_sb[:], wv.rearrange("(o p) c -> p o c", p=128))
        wo_sb = wsm.tile([128, 2, D], bf16)
        nc.sync.dma_start(wo_sb[:], wo.rearrange("(o p) c -> p o c", p=128))

        h_sb = acts.tile([128, NT, ROWS], bf16, tag="act8", name="h_sb")

        # ================= LN1 =================
        with tc.tile_pool(name="ps_st", bufs=2, space="PSUM") as ps_st:
            sum_ps = ps_st.tile([1, ROWS], f32, tag="st", name="sum_ps")
            sumsq_ps = ps_st.tile([1, ROWS], f32, tag="st", name="sumsq_ps")
            for kt in range(NT):
                xbf = tmp.tile([128, ROWS], bf16, tag="t2", name="xbf")
                nc.vector.tensor_copy(out=xbf[:], in_=xT_sb[:, kt])
                sq = tmp.tile([128, ROWS], bf16, tag="t2", name="sq")
                nc.vector.tensor_tensor(sq[:], xbf[:], xbf[:], OP.mult)
                nc.tensor.matmul(sum_ps[:], invd[:], xbf[:], start=(kt == 0), stop=(kt == NT - 1))
                nc.tensor.matmul(sumsq_ps[:], invd[:], sq[:], start=(kt == 0), stop=(kt == NT - 1))
            mean = rows.tile([1, ROWS], f32, tag="row", name="mean")
            nc.vector.tensor_copy(out=mean[:], in_=sum_ps[:])
            msq = rows.tile([1, ROWS], f32, tag="row", name="msq")
            nc.vector.tensor_tensor(msq[:], mean[:], mean[:], OP.mult)
            var = rows.tile([1, ROWS], f32, tag="row", name="var")
            nc.vector.tensor_tensor(var[:], sumsq_ps[:], msq[:], OP.subtract)
            std = rows.tile([1, ROWS], f32, tag="row", name="std")
            nc.scalar.activation(std[:], var[:], AF.Sqrt, bias=eps_sb[:])
            rstd = rows.tile([1, ROWS], f32, tag="row", name="rstd")
            nc.vector.reciprocal(rstd[:], std[:])
            m_bc = ps_st.tile([128, ROWS], f32, tag="st", name="m_bc")
            nc.tensor.matmul(m_bc[:], ones128[:], mean[:], start=True, stop=True)
            r_bc = ps_st.tile([128, ROWS], f32, tag="st", name="r_bc")
            nc.tensor.matmul(r_bc[:], ones128[:], rstd[:], start=True, stop=True)
            for kt in range(NT):
                t = tmp.tile([128, ROWS], f32, tag="t4", name="lnt")
                nc.vector.tensor_tensor(t[:], xT_sb[:, kt], m_bc[:], OP.subtract)
                nc.vector.tensor_tensor(h_sb[:, kt], t[:], r_bc[:], OP.mult)

        # ============ QKV projections + RoPE + head projections ============
        krot = acts.tile([128, NT, ROWS], bf16, tag="act8", name="krot")
        vbf = acts.tile([128, NT, ROWS], bf16, tag="act8", name="vbf")
        qrot = acts.tile([128, NT, ROWS], bf16, tag="act8", name="qrot")
        keyT_loc = const.tile([128, 2, ROWS], bf16)
        val_loc = const.tile([128, 4, HK], bf16)
        qhT = const.tile([128, 2, ROWS], bf16)
        in_cc = dram.tile([2 * HK * ROWS], bf16)     # keyh^T block then valh block
        out_cc = dram.tile([8 * HK * ROWS], bf16)

        def big_mm(psum_t, wname, mt, rhs_sb):
            for kt in range(NT):
                nc.tensor.matmul(psum_t[:], w_sb[wname][:, kt, mt * 128:(mt + 1) * 128],
                                 rhs_sb[:, kt], start=(kt == 0), stop=(kt == NT - 1))

        def rope(psl, bias_row, out_sb):
            # psl: list of 8 psum tiles ([128,ROWS] fp32); out_sb [128,NT,ROWS] bf16
            for pt in range(4):
                c = cos_sb[:, pt]
                s = sin_sb[:, pt]
                b1 = col(bias_row, pt)
                b2 = col(bias_row, pt + 4)
                t1 = tmp.tile([128, ROWS], bf16, tag="t2", name="r1")
                nc.vector.scalar_tensor_tensor(t1[:], psl[pt][:], b1, c, OP.add, OP.mult)
                t2 = tmp.tile([128, ROWS], bf16, tag="t2", name="r2")
                nc.vector.scalar_tensor_tensor(t2[:], psl[pt + 4][:], b2, s, OP.add, OP.mult)
                nc.vector.tensor_tensor(out_sb[:, pt], t1[:], t2[:], OP.subtract)
                t3 = tmp.tile([128, ROWS], bf16, tag="t2", name="r3")
                nc.vector.scalar_tensor_tensor(t3[:], psl[pt][:], b1, s, OP.add, OP.mult)
                t4 = tmp.tile([128, ROWS], bf16, tag="t2", name="r4")
                nc.vector.scalar_tensor_tensor(t4[:], psl[pt + 4][:], b2, c, OP.add, OP.mult)
                nc.vector.tensor_tensor(out_sb[:, pt + 4], t3[:], t4[:], OP.add)

        with tc.tile_pool(name="ps_mm", bufs=8, space="PSUM") as ps_mm:
            # k = rope(h @ w2 + b2)
            kps = []
            for mt in range(NT):
                p = ps_mm.tile([128, ROWS], f32, tag="mm", name=f"kps{mt}")
                big_mm(p, "w2", mt, h_sb)
                kps.append(p)
            rope(kps, 1, krot)
            # v = h @ w3 + b3
            for mt in range(NT):
                p = ps_mm.tile([128, ROWS], f32, tag="mm", name=f"vps{mt}")
                big_mm(p, "w3", mt, h_sb)
                nc.vector.tensor_scalar_add(vbf[:, mt], p[:], col(2, mt))
            # keyh^T = (v @ k_W + bk)^T   [256, 512]
            for mt2 in range(2):
                p = ps_mm.tile([128, ROWS], f32, tag="mm", name=f"keyps{mt2}")
                for kt in range(NT):
                    nc.tensor.matmul(p[:], wk_sb[:, kt, mt2 * 128:(mt2 + 1) * 128], vbf[:, kt],
                                     start=(kt == 0), stop=(kt == NT - 1))
                nc.vector.tensor_scalar_add(keyT_loc[:, mt2], p[:], bhc[:, 2 + mt2:3 + mt2])
            # valh = (k_rot @ v_W + bv)  natural [512, 256]
            for st in range(4):
                p = ps_mm.tile([128, HK], f32, tag="mm", name=f"valps{st}")
                for kt in range(NT):
                    nc.tensor.matmul(p[:], krot[:, kt, st * 128:(st + 1) * 128], wv_sb[:, kt],
                                     start=(kt == 0), stop=(kt == NT - 1))
                nc.vector.tensor_tensor(val_loc[:, st], p[:], bvb[:], OP.add)
            # ship local kv to DRAM and gather
            nc.sync.dma_start(in_cc[0:HK * ROWS].rearrange("(o p f) -> p o f", p=128, f=ROWS), keyT_loc[:])
            nc.sync.dma_start(in_cc[HK * ROWS:].rearrange("(st p c) -> p st c", p=128, c=HK), val_loc[:])
            nc.gpsimd.collective_compute(
                "AllGather", mybir.AluOpType.bypass,
                replica_groups=[[0, 1, 2, 3], [4, 5, 6, 7]],
                ins=[in_cc[:].opt()], outs=[out_cc[:].opt()],
            )
            # q = rope(h @ w1 + b1); qh^T = (q @ q_W + bq)^T
            qps = []
            for mt in range(NT):
                p = ps_mm.tile([128, ROWS], f32, tag="mm", name=f"qps{mt}")
                big_mm(p, "w1", mt, h_sb)
                qps.append(p)
            rope(qps, 0, qrot)
            for mt2 in range(2):
                p = ps_mm.tile([128, ROWS], f32, tag="mm", name=f"qhps{mt2}")
                for kt in range(NT):
                    nc.tensor.matmul(p[:], wq_sb[:, kt, mt2 * 128:(mt2 + 1) * 128], qrot[:, kt],
                                     start=(kt == 0), stop=(kt == NT - 1))
                nc.vector.tensor_scalar_add(qhT[:, mt2], p[:], bhc[:, mt2:mt2 + 1])

        # gathered kv into SBUF
        keyT_all = const.tile([128, 8, ROWS], bf16)   # [:, 2*cb+G, s_local]
        val_all = const.tile([128, 16, HK], bf16)     # [:, 4*cb+lt, c]
        BLK = 2 * HK * ROWS
        for cb in range(4):
            nc.sync.dma_start(keyT_all[:, 2 * cb:2 * cb + 2],
                              out_cc[cb * BLK:cb * BLK + HK * ROWS].rearrange("(o p f) -> p o f", p=128, f=ROWS))
            nc.sync.dma_start(val_all[:, 4 * cb:4 * cb + 4],
                              out_cc[cb * BLK + HK * ROWS:(cb + 1) * BLK].rearrange("(st p c) -> p st c", p=128, c=HK))

        # ================= attention =================
        aoT = const.tile([128, 2, ROWS], bf16)
        x1 = acts.tile([128, NT, ROWS], bf16, tag="act8", name="x1")
        with tc.tile_pool(name="ps_sc", bufs=4, space="PSUM") as ps_sc, \
             tc.tile_pool(name="ps_ao", bufs=2, space="PSUM") as ps_ao, \
             tc.tile_pool(name="ps_dn", bufs=1, space="PSUM") as ps_dn, \
             tc.tile_pool(name="ps_o", bufs=1, space="PSUM") as ps_o:
            for G in range(2):
                ao_ps = ps_ao.tile([128, ROWS], f32, tag="ao", name=f"ao{G}")
                dn_ps = ps_dn.tile([128, ROWS], f32, tag="dn", name=f"dn{G}")
                for stg in range(16):
                    cb, lt = stg // 4, stg % 4
                    scl = []
                    for hh in range(4):
                        sc = ps_sc.tile([128, ROWS], f32, tag="sc", name=f"sc{G}_{stg}_{hh}")
                        nc.tensor.matmul(sc[:],
                                         keyT_all[32 * hh:32 * hh + 32, 2 * cb + G, lt * 128:(lt + 1) * 128],
                                         qhT[32 * hh:32 * hh + 32, G],
                                         start=True, stop=True, tile_position=(32 * hh, 0))
                        scl.append(sc)
                    for hh in range(4):
                        E = epool.tile([128, ROWS], bf16, tag="E", name=f"E{G}_{stg}_{hh}")
                        nc.scalar.activation(E[:], scl[hh][:], AF.Exp)
                        nc.tensor.matmul(ao_ps[32 * hh:32 * hh + 32, :],
                                         val_all[:, 4 * cb + lt, 128 * G + 32 * hh:128 * G + 32 * hh + 32],
                                         E[:], start=(stg == 0), stop=(stg == 15), tile_position=(0, 32 * hh))
                        nc.tensor.matmul(dn_ps[32 * hh:32 * hh + 1, :], ones_bf[:], E[:],
                                         start=(stg == 0), stop=(stg == 15), tile_position=(0, 32 * hh))
                for hh in range(4):
                    dnr = rows.tile([1, ROWS], f32, tag="row", name=f"dnr{G}_{hh}")
                    nc.vector.reciprocal(dnr[:], dn_ps[32 * hh:32 * hh + 1, :])
                    rb_ps = ps_sc.tile([128, ROWS], f32, tag="sc", name=f"rbps{G}_{hh}")
                    nc.tensor.matmul(rb_ps[:], ones128[:], dnr[:], start=True, stop=True)
                    rb_sb = tmp.tile([128, ROWS], f32, tag="t4", name=f"rbsb{G}_{hh}")
                    nc.vector.tensor_copy(out=rb_sb[:], in_=rb_ps[:])
                    nc.vector.tensor_tensor(aoT[32 * hh:32 * hh + 32, G],
                                            ao_ps[32 * hh:32 * hh + 32, :],
                                            rb_sb[32 * hh:32 * hh + 32, :], OP.mult)
            # o_proj + residual (inside attention pool scope so G0 overlaps G1)
            for mt in range(NT):
                p = ps_o.tile([128, ROWS], f32, tag="o", name=f"ops{mt}")
                for G in range(2):
                    nc.tensor.matmul(p[:], wo_sb[:, G, mt * 128:(mt + 1) * 128], aoT[:, G],
                                     start=(G == 0), stop=(G == 1))
                nc.vector.scalar_tensor_tensor(x1[:, mt], p[:], col(3, mt), xT_sb[:, mt], OP.add, OP.add)

        # ============ SwiGLU + LN2 + out ============
        sg = acts.tile([128, NT, ROWS], bf16, tag="act8", name="sg")
        mm_sb = acts.tile([128, NT, ROWS], bf16, tag="act8", name="mm_sb")
        f_sb = const.tile([128, NT, ROWS], f32)
        for nm in ("wg", "wu", "wd"):
            w_sb[nm] = wts.tile([128, NT, D], bf16, tag="Wbig", name=f"W_{nm}")
            nc.sync.dma_start(w_sb[nm][:], wbig[nm].rearrange("(o p) c -> p o c", p=128))

        with tc.tile_pool(name="ps_mm2", bufs=8, space="PSUM") as ps2:
            for mt in range(NT):
                p = ps2.tile([128, ROWS], f32, tag="mm2", name=f"gps{mt}")
                big_mm(p, "wg", mt, x1)
                nc.scalar.activation(sg[:, mt], p[:], AF.Silu, bias=col(4, mt))
            for mt in range(NT):
                p = ps2.tile([128, ROWS], f32, tag="mm2", name=f"ups{mt}")
                big_mm(p, "wu", mt, x1)
                nc.vector.scalar_tensor_tensor(mm_sb[:, mt], p[:], col(5, mt), sg[:, mt], OP.add, OP.mult)

        with tc.tile_pool(name="ps_mm3", bufs=6, space="PSUM") as ps3, \
             tc.tile_pool(name="ps_st2", bufs=2, space="PSUM") as ps_st2:
            sum2 = ps_st2.tile([1, ROWS], f32, tag="st2", name="sum2")
            sumsq2 = ps_st2.tile([1, ROWS], f32, tag="st2", name="sumsq2")
            for mt in range(NT):
                p = ps3.tile([128, ROWS], f32, tag="mm3", name=f"dps{mt}")
                big_mm(p, "wd", mt, mm_sb)
                nc.vector.tensor_scalar_add(f_sb[:, mt], p[:], col(6, mt))
                fbf = tmp.tile([128, ROWS], bf16, tag="t2", name="fbf")
                nc.vector.tensor_copy(out=fbf[:], in_=f_sb[:, mt])
                sqf = tmp.tile([128, ROWS], bf16, tag="t2", name="sqf")
                nc.vector.tensor_tensor(sqf[:], fbf[:], fbf[:], OP.mult)
                nc.tensor.matmul(sum2[:], invd[:], fbf[:], start=(mt == 0), stop=(mt == NT - 1))
                nc.tensor.matmul(sumsq2[:], invd[:], sqf[:], start=(mt == 0), stop=(mt == NT - 1))
            mean2 = rows.tile([1, ROWS], f32, tag="row", name="mean2")
            nc.vector.tensor_copy(out=mean2[:], in_=sum2[:])
            msq2 = rows.tile([1, ROWS], f32, tag="row", name="msq2")
            nc.vector.tensor_tensor(msq2[:], mean2[:], mean2[:], OP.mult)
            var2 = rows.tile([1, ROWS], f32, tag="row", name="var2")
            nc.vector.tensor_tensor(var2[:], sumsq2[:], msq2[:], OP.subtract)
            std2 = rows.tile([1, ROWS], f32, tag="row", name="std2")
            nc.scalar.activation(std2[:], var2[:], AF.Sqrt, bias=eps_sb[:])
            rstd2 = rows.tile([1, ROWS], f32, tag="row", name="rstd2")
            nc.vector.reciprocal(rstd2[:], std2[:])
            m2_bc = ps_st2.tile([128, ROWS], f32, tag="st2", name="m2_bc")
            nc.tensor.matmul(m2_bc[:], ones128[:], mean2[:], start=True, stop=True)
            r2_bc = ps_st2.tile([128, ROWS], f32, tag="st2", name="r2_bc")
            nc.tensor.matmul(r2_bc[:], ones128[:], rstd2[:], start=True, stop=True)
            for mt in range(NT):
                t1 = tmp.tile([128, ROWS], f32, tag="t4", name="o1")
                nc.vector.tensor_tensor(t1[:], f_sb[:, mt], m2_bc[:], OP.subtract)
                t2 = tmp.tile([128, ROWS], f32, tag="t4", name="o2")
                nc.vector.scalar_tensor_tensor(t2[:], t1[:], col(7, mt), r2_bc[:], OP.mult, OP.mult)
                t3 = tmp.tile([128, ROWS], f32, tag="t4", name="o3")
                nc.vector.scalar_tensor_tensor(t3[:], t2[:], col(8, mt), f_sb[:, mt], OP.add, OP.add)
                nc.sync.dma_start(outT[mt * 128:(mt + 1) * 128, :], t3[:])
        ctx.close()

    nc.compile()
    return nc


def _prep_inputs(inputs):
    x = np.asarray(inputs["x"], np.float32)
    g1 = np.asarray(inputs["ln1_g"], np.float32)
    b1 = np.asarray(inputs["ln1_b"], np.float32)
    sc = 1.0 / math.sqrt(K)

    def fold(Wn, bn):
        W = np.asarray(inputs[Wn], np.float32)
        b = np.asarray(inputs[bn], np.float32)
        return (g1[:, None] * W).astype(BF16), (b + b1 @ W).astype(np.float32)

    w1, bw1 = fold("w1_W", "w1_b")
    w2, bw2 = fold("w2_W", "w2_b")
    w3, bw3 = fold("w3_W", "w3_b")
    wqv = (np.asarray(inputs["q_W"], np.float32).reshape(D, HK) * sc).astype(BF16)
    bq = (np.asarray(inputs["q_b"], np.float32).reshape(HK) * sc).astype(np.float32)
    wkv = np.asarray(inputs["k_W"], np.float32).reshape(D, HK).astype(BF16)
    bk = np.asarray(inputs["k_b"], np.float32).reshape(HK)
    wvv = np.asarray(inputs["v_W"], np.float32).reshape(D, HK).astype(BF16)
    bv = np.asarray(inputs["v_b"], np.float32).reshape(HK)
    wov = np.asarray(inputs["o_W"], np.float32).reshape(HK, D).astype(BF16)

    bcols = np.zeros((10, D), np.float32)
    bcols[0] = bw1
    bcols[1] = bw2
    bcols[2] = bw3
    bcols[3] = np.asarray(inputs["o_b"], np.float32)
    bcols[4] = np.asarray(inputs["gate_b"], np.float32)
    bcols[5] = np.asarray(inputs["up_b"], np.float32)
    bcols[6] = np.asarray(inputs["down_b"], np.float32)
    bcols[7] = np.asarray(inputs["ln2_g"], np.float32)
    bcols[8] = np.asarray(inputs["ln2_b"], np.float32)
    bcols[9, 0:HK] = bq
    bcols[9, HK:2 * HK] = bk
    bcols[9, 2 * HK:3 * HK] = bv

    wgv = np.asarray(inputs["gate_W"], np.float32).astype(BF16)
    wuv = np.asarray(inputs["up_W"], np.float32).astype(BF16)
    wdv = np.asarray(inputs["down_W"], np.float32).astype(BF16)

    pos = np.arange(S, dtype=np.float32)
    freq = np.power(10000.0, -np.arange(D // 2, dtype=np.float32) / (D // 2))
    ang = pos[:, None] * freq[None, :]  # [S, 512]
    cosA = np.cos(ang)
    sinA = np.sin(ang)

    in_maps = []
    for c in range(N_CORES):
        b = c // 4
        j = c % 4
        sl = slice(ROWS * j, ROWS * (j + 1))
        m = {
            "xT": np.ascontiguousarray(x[b, sl, :].T),
            "w1": w1, "w2": w2, "w3": w3,
            "wg": wgv, "wu": wuv, "wd": wdv,
            "wq": wqv, "wk": wkv, "wv": wvv, "wo": wov,
            "bcols": bcols,
            "cosT": np.ascontiguousarray(cosA[sl, :].T).astype(BF16),
            "sinT": np.ascontiguousarray(sinA[sl, :].T).astype(BF16),
        }
        in_maps.append(m)
    return in_maps


def kernel(**inputs):
    from concourse.bass_utils import run_bass_kernel_spmd

    if "nc" not in _CACHE:
        _CACHE["nc"] = _build_program()
    nc = _CACHE["nc"]
    in_maps = _prep_inputs(inputs)
    res = run_bass_kernel_spmd(nc, in_maps, list(range(N_CORES)))
    out = np.empty((B, S, D), np.float32)
    for c in range(N_CORES):
        b = c // 4
        j = c % 4
        out[b, ROWS * j:ROWS * (j + 1), :] = res.results[c]["outT"].T
    return out
